# revision 14
# baseline (speedup 1.0000x reference)
"""Trainium2 Bass kernel for nn_BertSelfAttention_ling (relative_key_query
position embeddings + char/word level biases).

Sharding: pure data-parallel over batch — 16 batches / 8 cores = 2 per core,
no collectives. Weights/embeddings replicated.

Per-core algorithm (index math validated by a numpy prototype):
  layout B scores: scoresT tiles [r(128 part), l(512 free)] per (b, h).
  - X^T via PE transpose; Q^T/K^T projections (head-dim on partitions),
    V natural; all matmul inputs bf16, fp32 PSUM accumulation.
  - q*pe / k*pe relative-position terms: windowed matmuls QE/KE [128, 639]
    per tile, round-tripped through DRAM scratch so the diagonal "shear"
    gather (QE[l, l-r+511]) becomes a contiguous-last-dim affine AP.
  - QEg (layout A [l, r]) tiles transpose-added into the scores PSUM via
    matmuls against an identity rhs; KEg ([r, l]) and the level bias are
    identity-matmul-added.
  - level bias: degree-7 Horner polynomial through the 8-entry tables
    (coefficients solved on host per call, shipped as an input tensor).
  - softmax: exp on ACT reading PSUM directly, attention_mask as the
    per-partition activation bias; 1/sqrt(D) folded into Wq / E / tables.
  - PV: ctxT = V_aug^T @ probsT with a ones column producing the softmax
    denominator; PE-transpose back to [l, d], normalize at ctx.
"""

import numpy as np

B, S, H, D = 16, 512, 12, 64
HID = H * D
P = 128
NLT = S // P            # 4 tiles of 128 along S
W = P + S - 1           # 639 window width
BPC = B // 8            # batches per core = 2
NMAX = 512              # psum bank limit for f32 matmul output

_CACHE = {}


def _build_program(coefs):
    import concourse.bass as bass
    import concourse.bacc as bacc
    import concourse.mybir as mybir
    from concourse.tile import TileContext
    from concourse.masks import make_identity

    dt = mybir.dt
    AF = mybir.ActivationFunctionType
    OP = mybir.AluOpType

    nc = bacc.Bacc(None, target_bir_lowering=False, debug=False)

    # ---------------- external tensors ----------------
    x_in = nc.dram_tensor("x", [BPC, S, HID], dt.float32, kind="ExternalInput")
    cmat_in = nc.dram_tensor("cmat", [BPC, S, S], dt.int32, kind="ExternalInput")
    wmat_in = nc.dram_tensor("wmat", [BPC, S, S], dt.int32, kind="ExternalInput")
    mask_in = nc.dram_tensor("maskr", [BPC, S], dt.float32, kind="ExternalInput")
    wqt_in = nc.dram_tensor("wqt", [HID, HID], dt.bfloat16, kind="ExternalInput")
    wkt_in = nc.dram_tensor("wkt", [HID, HID], dt.bfloat16, kind="ExternalInput")
    wvt_in = nc.dram_tensor("wvt", [HID, HID], dt.bfloat16, kind="ExternalInput")
    bq_in = nc.dram_tensor("bq2", [1, HID], dt.float32, kind="ExternalInput")
    bk_in = nc.dram_tensor("bk2", [1, HID], dt.float32, kind="ExternalInput")
    bv_in = nc.dram_tensor("bv2", [1, HID], dt.float32, kind="ExternalInput")
    etr_in = nc.dram_tensor("etrev", [D, 2 * S - 1], dt.bfloat16, kind="ExternalInput")
    et8_in = nc.dram_tensor("et8", [D, 2 * S - 1], dt.bfloat16, kind="ExternalInput")
    out_dram = nc.dram_tensor("out", [BPC, S, HID], dt.float32, kind="ExternalOutput")
    # Horner coefficients baked as immediates (program cached per table hash):
    # coefs[0] = char a7..a1 + [a0c+a0w], coefs[1] = word a7..a1 + [0]

    with TileContext(nc) as tc:
        with (
            tc.tile_pool(name="const", bufs=1) as constp,
            tc.tile_pool(name="persist", bufs=1) as persist,
            tc.tile_pool(name="ld", bufs=2) as ldp,
            tc.tile_pool(name="win_sb", bufs=3) as win_sb,
            tc.tile_pool(name="diag_sb", bufs=3) as diag_sb,
            tc.tile_pool(name="probs_sb", bufs=3) as probs_sb,
            tc.tile_pool(name="misc_sb", bufs=2) as misc_sb,
            # PSUM: 8 banks total: win 2x2 + sc 2x1 + ctxT 1 + small 1 = 8.
            # tp/proj/ctxt share the "small" pool slot (max 2KB/partition).
            tc.tile_pool(name="small_ps", bufs=1, space="PSUM") as small_ps,
            tc.tile_pool(name="win_ps", bufs=2, space="PSUM") as win_ps,
            tc.tile_pool(name="sc_ps", bufs=2, space="PSUM") as sc_ps,
            tc.tile_pool(name="ctxT_ps", bufs=1, space="PSUM") as ctxT_ps,
            tc.tile_pool(name="dram", bufs=28, space="DRAM") as dramp,
        ):
            # ------------ constants ------------
            ident = constp.tile([P, P], dt.bfloat16)
            make_identity(nc, ident[:])
            identf = constp.tile([P, P], dt.float32)
            make_identity(nc, identf[:])

            etr = constp.tile([P, 2 * S - 1], dt.bfloat16)   # two stacked copies
            nc.sync.dma_start(etr[0:D, :], etr_in[:, :])
            nc.sync.dma_start(etr[D:2 * D, :], etr_in[:, :])
            et8 = constp.tile([P, 2 * S - 1], dt.bfloat16)
            nc.sync.dma_start(et8[0:D, :], et8_in[:, :])
            nc.sync.dma_start(et8[D:2 * D, :], et8_in[:, :])

            # per-partition bias layouts: bq_sb[p, ot] = bq[ot*128 + p]
            bq_sb = constp.tile([P, 6], dt.float32)
            bk_sb = constp.tile([P, 6], dt.float32)
            for ot in range(6):
                nc.sync.dma_start(bq_sb[:, ot:ot + 1],
                                  bq_in[0, ot * P:(ot + 1) * P].unsqueeze(1))
                nc.sync.dma_start(bk_sb[:, ot:ot + 1],
                                  bk_in[0, ot * P:(ot + 1) * P].unsqueeze(1))
            # bv replicated to all partitions (values vary along free dim)
            bv_sb = constp.tile([P, HID], dt.float32)
            nc.sync.dma_start(bv_sb[:], bv_in[:, :].to_broadcast([P, HID]))

            mask_sb = constp.tile([P, BPC * NLT], dt.float32)
            for b in range(BPC):
                for rt in range(NLT):
                    nc.sync.dma_start(
                        mask_sb[:, b * NLT + rt: b * NLT + rt + 1],
                        mask_in[b, rt * P:(rt + 1) * P].unsqueeze(1))

            # weight tiles: wqt [i, o] natural; 6 partition tiles of [128, 768]
            wq_sb = persist.tile([P, 6 * HID], dt.bfloat16)
            wk_sb = persist.tile([P, 6 * HID], dt.bfloat16)
            wv_sb = persist.tile([P, 6 * HID], dt.bfloat16)
            for it in range(6):
                nc.sync.dma_start(wq_sb[:, it * HID:(it + 1) * HID],
                                  wqt_in[it * P:(it + 1) * P, :])
                nc.sync.dma_start(wk_sb[:, it * HID:(it + 1) * HID],
                                  wkt_in[it * P:(it + 1) * P, :])
                nc.sync.dma_start(wv_sb[:, it * HID:(it + 1) * HID],
                                  wvt_in[it * P:(it + 1) * P, :])

            # ------------ level bias for BOTH batches (Horner, packed) ----
            # computed up-front so DVE's polynomial work overlaps the PE-side
            # projections and window matmuls
            biasA = {}
            for b in range(BPC):
                biasA_t = persist.tile([P, NLT * S], dt.bfloat16, tag=f"biasA{b}")
                biasA[b] = biasA_t
            for lt in range(NLT):
                cm = ldp.tile([P, BPC * S], dt.bfloat16, tag="cm")
                wm = ldp.tile([P, BPC * S], dt.bfloat16, tag="wm")
                for b in range(BPC):
                    nc.gpsimd.dma_start(cm[:, b * S:(b + 1) * S],
                                        cmat_in[b, lt * P:(lt + 1) * P, :])
                    nc.gpsimd.dma_start(wm[:, b * S:(b + 1) * S],
                                        wmat_in[b, lt * P:(lt + 1) * P, :])
                eng = nc.vector  # STT=TensorScalarPtr is DVE-only
                # f32 intermediates: the interpolating polynomial has large
                # coefficient cancellation; bf16 intermediates lose ~0.02.
                yc = misc_sb.tile([P, BPC * S], dt.float32, tag="yc")
                yw = misc_sb.tile([P, BPC * S], dt.float32, tag="yw")
                eng.tensor_scalar_mul(yc[:], cm[:], float(coefs[0][0]))
                eng.tensor_scalar_mul(yw[:], wm[:], float(coefs[1][0]))
                for k in range(1, 7):
                    eng.scalar_tensor_tensor(yc[:], yc[:], float(coefs[0][k]),
                                             cm[:], op0=OP.add, op1=OP.mult)
                    eng.scalar_tensor_tensor(yw[:], yw[:], float(coefs[1][k]),
                                             wm[:], op0=OP.add, op1=OP.mult)
                for b in range(BPC):
                    eng.scalar_tensor_tensor(
                        biasA[b][:, lt * S:(lt + 1) * S],
                        yc[:, b * S:(b + 1) * S], float(coefs[0][7]),
                        yw[:, b * S:(b + 1) * S], op0=OP.add, op1=OP.add)

            qe_scr = {}
            ke_scr = {}
            for b in range(BPC):
                # ------------ X^T ------------
                xt = persist.tile([P, 6 * S], dt.bfloat16, tag=f"xt{b}")
                for st in range(NLT):
                    xb = ldp.tile([P, HID], dt.bfloat16, tag="xb")
                    nc.gpsimd.dma_start(xb[:], x_in[b, st * P:(st + 1) * P, :])
                    for it in range(6):
                        pt = small_ps.tile([P, P], dt.bfloat16, tag="setup")
                        nc.tensor.transpose(pt[:], xb[:, it * P:(it + 1) * P],
                                            ident[:])
                        dst = xt[:, it * S + st * P: it * S + (st + 1) * P]
                        if (st + it) % 2 == 0:
                            nc.vector.tensor_copy(dst, pt[:])
                        else:
                            nc.scalar.activation(dst, pt[:], AF.Copy)

                # ------------ projections ------------
                qt = persist.tile([P, 6 * S], dt.bfloat16, tag=f"qt{b}")
                kt = persist.tile([P, 6 * S], dt.bfloat16, tag=f"kt{b}")
                for ot in range(6):
                    for wsb, bsb, dst in ((wq_sb, bq_sb, qt), (wk_sb, bk_sb, kt)):
                        ps = small_ps.tile([P, S], dt.float32, tag="setup")
                        for it in range(6):
                            nc.tensor.matmul(
                                ps[:],
                                wsb[:, it * HID + ot * P: it * HID + (ot + 1) * P],
                                xt[:, it * S:(it + 1) * S],
                                start=(it == 0), stop=(it == 5))
                        nc.vector.tensor_scalar_add(
                            dst[:, ot * S:(ot + 1) * S], ps[:], bsb[:, ot:ot + 1])

                # V_aug: per rt block [128, 12*65] bf16 (64 cols V + ones col)
                vaug = persist.tile([P, NLT * (H * 65)], dt.bfloat16, tag=f"va{b}")
                for rt in range(NLT):
                    base = rt * (H * 65)
                    for oc in range(2):  # o chunks of 384 = 6 heads
                        ps = small_ps.tile([P, 384], dt.float32, tag="setup")
                        for it in range(6):
                            nc.tensor.matmul(
                                ps[:],
                                xt[:, it * S + rt * P: it * S + (rt + 1) * P],
                                wv_sb[:, it * HID + oc * 384:
                                      it * HID + (oc + 1) * 384],
                                start=(it == 0), stop=(it == 5))
                        dst = vaug[:, base + oc * 6 * 65: base + (oc + 1) * 6 * 65] \
                            .rearrange("p (h c) -> p h c", c=65)[:, :, 0:64]
                        bvb = bv_sb[:, oc * 384:(oc + 1) * 384] \
                            .rearrange("p (h c) -> p h c", c=64)
                        nc.vector.scalar_tensor_tensor(
                            dst, ps[:].rearrange("p (h c) -> p h c", c=64),
                            1.0, bvb, op0=OP.mult, op1=OP.add)
                    ones = vaug[:, base: base + H * 65] \
                        .rearrange("p (h c) -> p h c", c=65)[:, :, 64:65]
                    nc.gpsimd.memset(ones, 1.0)

                # ------------ biasT transposes (batched evicts) ------------
                biasT = persist.tile([P, NLT * S], dt.bfloat16, tag=f"biasT{b}")
                for rt in range(NLT):
                    pt = small_ps.tile([P, S], dt.bfloat16, tag="setup")
                    for lt in range(NLT):
                        nc.tensor.transpose(
                            pt[:, lt * P:(lt + 1) * P],
                            biasA[b][:, lt * S + rt * P: lt * S + (rt + 1) * P],
                            ident[:])
                    dst = biasT[:, rt * S:(rt + 1) * S]
                    if rt % 2 == 0:
                        nc.vector.tensor_copy(dst, pt[:])
                    else:
                        nc.scalar.activation(dst, pt[:], AF.Copy)

                # ------------ phase A: all heads' QE/KE windows ------------
                for h in range(H):
                    po = (h % 2) * D
                    ot = h // 2
                    qh = qt[po:po + D, ot * S:(ot + 1) * S]
                    kh = kt[po:po + D, ot * S:(ot + 1) * S]
                    qed_t = dramp.tile([NLT, P, W], dt.bfloat16, tag="qed")
                    ked_t = dramp.tile([NLT, P, W], dt.bfloat16, tag="ked")
                    qe_scr[(b, h)] = qed_t
                    ke_scr[(b, h)] = ked_t
                    for di, (dst_dram, src, ee) in enumerate(
                            ((qe_scr[(b, h)], qh, etr), (ke_scr[(b, h)], kh, et8))):
                        wins = win_sb.tile([P, NLT * W], dt.bfloat16, tag="wins")
                        for t in range(NLT):
                            lo = 384 - P * t
                            ps = win_ps.tile([P, W], dt.float32, tag="winps")
                            lhsT = src[:, t * P:(t + 1) * P]
                            nc.tensor.matmul(
                                ps[:, 0:NMAX], lhsT,
                                ee[po:po + D, lo:lo + NMAX],
                                start=True, stop=True)
                            nc.tensor.matmul(
                                ps[:, NMAX:W], lhsT,
                                ee[po:po + D, lo + NMAX:lo + W],
                                start=True, stop=True)
                            dst = wins[:, t * W:(t + 1) * W]
                            if (di + t + h) % 2 == 0:
                                nc.vector.tensor_copy(dst, ps[:])
                            else:
                                nc.scalar.activation(dst, ps[:], AF.Copy)
                        nc.sync.dma_start(
                            dst_dram[:].rearrange("t p w -> p t w"), wins[:])

                # ------------ phase B: all heads' scores/softmax/PV ----------
                ctx_all = persist.tile([P, NLT * H * 65], dt.float32,
                                       tag=f"ctxall{b}")
                for h in range(H):
                    po = (h % 2) * D
                    ot = h // 2
                    qh = qt[po:po + D, ot * S:(ot + 1) * S]
                    kh = kt[po:po + D, ot * S:(ot + 1) * S]

                    # diagonal shear reads; scratch flat [t, p, w]:
                    #   elem (p, t, j) at 127 + t*P*W + p*(W-1) + j
                    qeg = diag_sb.tile([P, NLT * S], dt.bfloat16, tag="qeg")
                    keg = diag_sb.tile([P, NLT * S], dt.bfloat16, tag="keg")
                    for sb, dr in ((qeg, qe_scr[(b, h)]), (keg, ke_scr[(b, h)])):
                        src_ap = dr[:]
                        diag = bass.AP(
                            src_ap.tensor, src_ap.offset + 127,
                            [[W - 1, P], [P * W, NLT], [1, S]])
                        nc.sync.dma_start(
                            sb[:].rearrange("p (t j) -> p t j", j=S), diag)

                    ctxT = ctxT_ps.tile([D + 1, S], dt.float32, tag="ctxT")
                    for rt in range(NLT):
                        sc = sc_ps.tile([P, S], dt.float32, tag="sc")
                        nc.tensor.matmul(sc[:], kh[:, rt * P:(rt + 1) * P], qh,
                                         start=True, stop=False,
                                         skip_group_check=True)
                        for lt in range(NLT):
                            nc.tensor.matmul(
                                sc[:, lt * P:(lt + 1) * P],
                                qeg[:, lt * S + rt * P: lt * S + (rt + 1) * P],
                                ident[:], start=False, stop=False,
                                skip_group_check=True)
                        nc.tensor.matmul(sc[:], ident[:],
                                         keg[:, rt * S:(rt + 1) * S],
                                         start=False, stop=False,
                                         skip_group_check=True)
                        nc.tensor.matmul(sc[:], ident[:],
                                         biasT[:, rt * S:(rt + 1) * S],
                                         start=False, stop=True,
                                         skip_group_check=True)
                        probs = probs_sb.tile([P, S], dt.bfloat16, tag="probs")
                        nc.scalar.activation(
                            probs[:], sc[:], AF.Exp,
                            bias=mask_sb[:, b * NLT + rt: b * NLT + rt + 1],
                            scale=1.0)
                        nc.tensor.matmul(
                            ctxT[:],
                            vaug[:, rt * H * 65 + h * 65:
                                 rt * H * 65 + (h + 1) * 65],
                            probs[:], start=(rt == 0), stop=(rt == 3))

                    # ctxT evict + batched transpose back to [l, d]
                    ctxT_sb = misc_sb.tile([D + 1, S], dt.float32, tag="ctxTsb")
                    nc.vector.tensor_copy(ctxT_sb[:], ctxT[:])
                    pt2 = small_ps.tile([P, NLT * (D + 1)], dt.float32,
                                        tag="setup")
                    for lt in range(NLT):
                        nc.tensor.matmul(pt2[:, lt * (D + 1):(lt + 1) * (D + 1)],
                                         ctxT_sb[:, lt * P:(lt + 1) * P],
                                         identf[0:D + 1, 0:D + 1],
                                         is_transpose=True)
                    dst3 = ctx_all[:].rearrange(
                        "p (lt h c) -> p lt h c", lt=NLT, c=65)[:, :, h, :]
                    if h % 2 == 0:
                        nc.scalar.activation(
                            dst3, pt2[:].rearrange("p (lt c) -> p lt c", c=65),
                            AF.Copy)
                    else:
                        nc.vector.tensor_copy(
                            dst3, pt2[:].rearrange("p (lt c) -> p lt c", c=65))

                # ------------ normalize + store ------------
                for lt in range(NLT):
                    blk3 = ctx_all[:, lt * H * 65:(lt + 1) * H * 65] \
                        .rearrange("p (h c) -> p h c", c=65)
                    recip = misc_sb.tile([P, H], dt.float32, tag="recip")
                    nc.vector.reciprocal(recip[:], blk3[:, :, 64])
                    outsb = misc_sb.tile([P, HID], dt.float32, tag="outsb")
                    nc.vector.tensor_tensor(
                        outsb[:].rearrange("p (h c) -> p h c", c=64),
                        blk3[:, :, 0:64],
                        recip[:].unsqueeze(2).to_broadcast([P, H, 64]),
                        op=OP.mult)
                    nc.sync.dma_start(out_dram[b, lt * P:(lt + 1) * P, :],
                                      outsb[:])

    nc.finalize()
    return nc


def _get_program(coefs):
    key = ("nc", tuple(coefs[0]), tuple(coefs[1]))
    if key not in _CACHE:
        _CACHE[key] = _build_program(coefs)
    return _CACHE[key]


def kernel(**inputs):
    import ml_dtypes
    from concourse.bass_utils import run_bass_kernel_spmd

    bf16 = ml_dtypes.bfloat16
    f32 = np.float32

    hs = np.ascontiguousarray(np.asarray(inputs["hidden_states"], f32))
    am = np.asarray(inputs["attention_mask"], f32).reshape(B, S)
    cm = np.asarray(inputs["character_level_matrix"])
    wm = np.asarray(inputs["word_level_matrix"])
    cm = np.ascontiguousarray(cm.astype(np.int32) if cm.dtype != np.int32 else cm)
    wm = np.ascontiguousarray(wm.astype(np.int32) if wm.dtype != np.int32 else wm)

    Wq = np.asarray(inputs["Wq"], f32)
    Wk = np.asarray(inputs["Wk"], f32)
    Wv = np.asarray(inputs["Wv"], f32)
    bq = np.asarray(inputs["bq"], f32)
    bk = np.asarray(inputs["bk"], f32)
    bv = np.asarray(inputs["bv"], f32)
    E = np.asarray(inputs["dist_emb"], f32)
    chtab = np.asarray(inputs["char_emb"], f32)[:, 0] / 16.0
    wdtab = np.asarray(inputs["word_emb"], f32)[:, 0] / 16.0

    wqt = np.ascontiguousarray((Wq / 8.0).T).astype(bf16)
    wkt = np.ascontiguousarray(Wk.T).astype(bf16)
    wvt = np.ascontiguousarray(Wv.T).astype(bf16)
    etrev = np.ascontiguousarray(E[::-1].T).astype(bf16)
    et8 = np.ascontiguousarray((E / 8.0).T).astype(bf16)

    # exact degree-7 polynomial through the 8 table points (float64 solve),
    # baked into the program as immediates (fp32-rounded for cache keying)
    V = np.vander(np.arange(8, dtype=np.float64), 8, increasing=True)
    cc = np.linalg.solve(V, chtab.astype(np.float64))
    cw = np.linalg.solve(V, wdtab.astype(np.float64))
    coefs = (
        np.concatenate([cc[7:0:-1], [cc[0] + cw[0]]]).astype(f32),
        np.concatenate([cw[7:0:-1], [0.0]]).astype(f32),
    )

    nc = _get_program(coefs)
    in_maps = []
    for c in range(8):
        sl = slice(c * BPC, (c + 1) * BPC)
        in_maps.append({
            "x": hs[sl], "cmat": cm[sl], "wmat": wm[sl], "maskr": am[sl],
            "wqt": wqt, "wkt": wkt, "wvt": wvt,
            "bq2": (bq / 8.0).reshape(1, HID), "bk2": bk.reshape(1, HID),
            "bv2": bv.reshape(1, HID),
            "etrev": etrev, "et8": et8,
        })
    res = run_bass_kernel_spmd(nc, in_maps, core_ids=list(range(8)))
    _CACHE["last_result"] = res
    out = np.concatenate([res.results[c]["out"] for c in range(8)], axis=0)
    return np.ascontiguousarray(out.astype(np.float32))
